# revision 28
# baseline (speedup 1.0000x reference)
"""NNUE-style embedding-lookup + tiny-MLP kernel for Trainium2 (8 NeuronCores).

Data-parallel over the batch dim: each of the 8 cores handles 2048 of the
16384 batch positions; the 50 MB embedding table and MLP weights are
replicated per core.

Per-core device program:
  1. dma_gather (SWDGE batched gather) pulls the 32 active-feature embedding
     rows per batch position from HBM into SBUF, 2064 rows per call, rotated
     across 4 SWDGE queues so descriptor generation runs on all Q7 core
     pairs concurrently. Indices are pre-biased by -16384 so they fit the
     gather's signed-int16 index format (the Q7 descriptor generator
     sign-extends and does a signed multiply-accumulate onto the base
     address, so the base is advanced by +16384 rows).
  2. TensorE accumulates the gathered slots (one feature x 128 batches each)
     into PSUM with float32r identity matmuls, two slots (512 columns) per
     matmul; a DVE add folds the even/odd halves into x[128b, 256].
  3. TensorE transposes x, then runs the 256->32->32->1 MLP with ScalarE
     handling bias+ReLU/Tanh.
"""

import numpy as np

INPUT_DIM = 49152
E = 256              # embedding width (1 KB rows)
BATCH = 16384
F = 32               # active features per position
N_CORES = 8
B_CORE = BATCH // N_CORES          # 2048 batch positions per core
BIAS = 16384                       # index bias for int16 gather
CHUNK_REAL = 4096                  # gathered rows per call (32 feats x 128 batches)
PAD = 16                           # tail pad (keeps last index >= 0)
CHUNK = CHUNK_REAL + PAD           # 2064
CW = CHUNK // 16                   # 129 idx cols per chunk in [16, .] layout
CW_STRIDE = 160                    # padded col stride: 320 B, 64 B-aligned
NCH = (B_CORE * F) // CHUNK_REAL   # 32 full-size gather calls (before tail split)
NT = B_CORE // 128                 # 16 t-blocks (128 batches each)
S_REAL = CHUNK_REAL // 128         # 16 real slots per full call
S = (CHUNK + 127) // 128           # 17 slots incl. the pad slot


def _call_table():
    """Per-t-block gather calls: (col_offset, n_cols_used, num_idxs, n_real_slots).

    t-blocks 0..NT-2 use two 2048-row calls; the last t-block is split into
    four 512-row calls so the tail data-drain and compute pipeline finely.
    """
    calls = []
    col = 0
    for t in range(NT):
        sizes = [CHUNK_REAL] if CHUNK_REAL == 4096 else [CHUNK_REAL, CHUNK_REAL]
        tcalls = []
        for sz in sizes:
            n_idx = sz + PAD
            cw = n_idx // 16
            stride = ((cw + 31) // 32) * 32  # 64 B-aligned call starts
            tcalls.append((col, cw, n_idx, sz // 128))
            col += stride
        calls.append(tcalls)
    return calls, col


CALLS, IDX_COLS = _call_table()

_nc_cache = None


def _build():
    import os
    import concourse.bacc as bacc
    import concourse.mybir as mybir
    import concourse.tile as tile

    stage = os.environ.get("KERNEL_STAGE", "full")
    nt = int(os.environ.get("KERNEL_NT", str(NT)))
    bf16 = os.environ.get("KERNEL_BF16", "0") == "1"

    f32 = mybir.dt.float32
    f32r = mybir.dt.float32r
    i16 = mybir.dt.int16
    AF = mybir.ActivationFunctionType

    gdt = mybir.dt.bfloat16 if bf16 else f32r
    edt = mybir.dt.bfloat16 if bf16 else f32
    nc = bacc.Bacc(None, target_bir_lowering=False, debug=False, num_swdge_queues=4)
    emb = nc.dram_tensor("emb", [INPUT_DIM, E], edt, kind="ExternalInput")
    idx = nc.dram_tensor("idx", [128, IDX_COLS], i16, kind="ExternalInput")
    ident = nc.dram_tensor("ident", [128, 128], f32, kind="ExternalInput")
    w1t = nc.dram_tensor("w1t", [128, 2, 32], f32, kind="ExternalInput")
    b1 = nc.dram_tensor("b1", [32, 1], f32, kind="ExternalInput")
    w2l = nc.dram_tensor("w2l", [32, 32], f32, kind="ExternalInput")
    b2 = nc.dram_tensor("b2", [32, 1], f32, kind="ExternalInput")
    w3l = nc.dram_tensor("w3l", [32, 1], f32, kind="ExternalInput")
    b3 = nc.dram_tensor("b3", [1, 1], f32, kind="ExternalInput")
    out = nc.dram_tensor("out", [1, B_CORE], f32, kind="ExternalOutput")

    with tile.TileContext(nc) as tc:
        with (
            tc.tile_pool(name="const", bufs=1) as cpool,
            tc.tile_pool(name="g", bufs=(8 if CHUNK_REAL == 2048 else 5)) as gpool,
            tc.tile_pool(name="xs", bufs=2) as xspool,
            tc.tile_pool(name="xts", bufs=2) as xtspool,
            tc.tile_pool(name="hs", bufs=4) as hspool,
            tc.tile_pool(name="xp", bufs=2, space="PSUM") as xppool,
            tc.tile_pool(name="xtp", bufs=2, space="PSUM") as xtppool,
            tc.tile_pool(name="mp", bufs=4, space="PSUM") as mppool,
        ):
            idx_t = cpool.tile([128, IDX_COLS], i16)
            idx_slice = (IDX_COLS // 8 + 31) // 32 * 32
            for k in range(8):
                lo = k * idx_slice
                hi = min((k + 1) * idx_slice, IDX_COLS)
                if lo < hi:
                    nc.sync.dma_start(idx_t[:, lo:hi], idx[:, lo:hi])
            id_t = cpool.tile([128, 128], f32)
            nc.sync.dma_start(id_t[:], ident[:])
            idr_t = cpool.tile([128, 128], gdt)
            if bf16:
                nc.gpsimd.dma_start(idr_t[:], ident[:])
            else:
                nc.sync.dma_start(idr_t[:], ident[:].bitcast(f32r))
            w1t_t = cpool.tile([128, 2, 32], f32)
            nc.sync.dma_start(w1t_t[:], w1t[:])
            b1_t = cpool.tile([32, 1], f32)
            nc.sync.dma_start(b1_t[:], b1[:])
            w2l_t = cpool.tile([32, 32], f32)
            nc.sync.dma_start(w2l_t[:], w2l[:])
            b2_t = cpool.tile([32, 1], f32)
            nc.sync.dma_start(b2_t[:], b2[:])
            w3l_t = cpool.tile([32, 1], f32)
            nc.sync.dma_start(w3l_t[:], w3l[:])
            b3_t = cpool.tile([1, 1], f32)
            nc.sync.dma_start(b3_t[:], b3[:])
            out_t = cpool.tile([1, B_CORE], f32)

            qn = 0
            for t in range(nt):
                xp = xppool.tile([128, 2, E], f32, tag="xp")
                tcalls = CALLS[t]
                n_mm = sum(nrs // 2 for _, _, _, nrs in tcalls)
                mm = 0
                for col, cw, n_idx, n_real_slots in tcalls:
                    n_slots = (n_idx + 127) // 128
                    g = gpool.tile([128, S, E], gdt, tag="g")
                    nc.gpsimd.dma_gather(
                        g[:, :n_slots, :],
                        emb[BIAS:, :].bitcast(gdt),
                        idx_t[:, col : col + cw],
                        n_idx,
                        n_idx,
                        E,
                        single_packet=False,
                        queue_num=qn % 4,
                    )
                    qn += 1
                    if stage == "gather":
                        nc.sync.dma_start(
                            out[:, (qn % 8) * 128 : (qn % 8) * 128 + 128],
                            g[0:1, 0, :128].bitcast(f32),
                        )
                        continue
                    for sp in range(n_real_slots // 2):
                        nc.tensor.matmul(
                            xp[:],
                            lhsT=idr_t[:],
                            rhs=g[:, 2 * sp : 2 * sp + 2, :],
                            start=(mm == 0),
                            stop=(mm == n_mm - 1),
                        )
                        mm += 1
                if stage == "gather":
                    continue
                x_sb = xspool.tile([128, E], f32, tag="xs")
                nc.vector.tensor_reduce(
                    out=x_sb[:],
                    in_=xp[:].rearrange("p h e -> p e h"),
                    axis=mybir.AxisListType.X,
                    op=mybir.AluOpType.add,
                )
                if stage == "reduce":
                    nc.sync.dma_start(
                        out[:, (t % 8) * 256 : (t % 8) * 256 + 256], x_sb[0:1, :]
                    )
                    continue
                xt_p = xtppool.tile([128, 2, 128], f32, tag="xtp")
                for h in range(2):
                    nc.tensor.transpose(
                        xt_p[:, h, :], x_sb[:, 128 * h : 128 * (h + 1)], id_t[:]
                    )
                xt_sb = xtspool.tile([128, 2, 128], f32, tag="xts")
                nc.vector.tensor_copy(xt_sb[:], xt_p[:])
                h1p = mppool.tile([32, 128], f32, tag="mp")
                for h in range(2):
                    nc.tensor.matmul(
                        h1p[:],
                        lhsT=w1t_t[:, h, :],
                        rhs=xt_sb[:, h, :],
                        start=(h == 0),
                        stop=(h == 1),
                    )
                h1s = hspool.tile([32, 128], f32, tag="hs")
                nc.scalar.activation(h1s[:], h1p[:], AF.Relu, bias=b1_t[:])
                h2p = mppool.tile([32, 128], f32, tag="mp")
                nc.tensor.matmul(h2p[:], lhsT=w2l_t[:], rhs=h1s[:], start=True, stop=True)
                h2s = hspool.tile([32, 128], f32, tag="hs")
                nc.scalar.activation(h2s[:], h2p[:], AF.Relu, bias=b2_t[:])
                yp = mppool.tile([1, 128], f32, tag="mp")
                nc.tensor.matmul(yp[:], lhsT=w3l_t[:], rhs=h2s[:], start=True, stop=True)
                nc.scalar.activation(
                    out_t[:, 128 * t : 128 * (t + 1)], yp[:], AF.Tanh, bias=b3_t[:]
                )
            if stage == "full":
                nc.sync.dma_start(out[:], out_t[:])
    nc.compile()
    return nc


def _get_nc():
    global _nc_cache
    if _nc_cache is None:
        _nc_cache = _build()
    return _nc_cache


def _prep_indices(shard: np.ndarray) -> np.ndarray:
    """[F, B_CORE] int -> [128, NCH*CW_STRIDE] int16 device layout.

    Position order p = t*4096 + f*128 + (b % 128), t = b // 128: each gather
    slot (128 consecutive positions) holds one feature for 128 batches, so
    the feature-sum is a PSUM accumulation over the slots. Each 2064-index
    gather call covers 16 features; indices are biased by -BIAS, padded with
    16 zeros (row BIAS, keeps the tail non-negative so the Q7 truncation
    loop is a no-op), laid out [16, CW] wrapped, replicated across the 8
    Q7 core groups, and 64 B-aligned per call.
    """
    arr = shard.reshape(F, NT, 128)  # [f, t, b_in]
    biased = arr.transpose(1, 0, 2).astype(np.int64) - BIAS  # [t, f, b_in]
    outa = np.zeros((128, IDX_COLS), np.int16)
    for t in range(NT):
        flat = biased[t].reshape(-1)  # 4096 positions, feature-major
        pos = 0
        for col, cw, n_idx, n_real_slots in CALLS[t]:
            n_real = n_real_slots * 128
            lst = np.zeros(n_idx, np.int64)
            lst[:n_real] = flat[pos : pos + n_real]
            pos += n_real
            lay = lst.reshape(cw, 16).T  # [16, cw]
            outa[:, col : col + cw] = np.tile(lay, (8, 1))
    return outa


def kernel(**inputs) -> np.ndarray:
    import os
    from concourse.bass_utils import run_bass_kernel_spmd

    indices = np.asarray(inputs["indices"])
    emb = np.ascontiguousarray(np.asarray(inputs["emb"], dtype=np.float32))
    if os.environ.get("KERNEL_BF16", "0") == "1":
        import ml_dtypes

        emb = emb.astype(ml_dtypes.bfloat16)
    w1 = np.asarray(inputs["w1"], dtype=np.float32)
    b1 = np.asarray(inputs["b1"], dtype=np.float32)
    w2 = np.asarray(inputs["w2"], dtype=np.float32)
    b2 = np.asarray(inputs["b2"], dtype=np.float32)
    w3 = np.asarray(inputs["w3"], dtype=np.float32)
    b3 = np.asarray(inputs["b3"], dtype=np.float32)

    ident = np.eye(128, dtype=np.float32)
    w1t_dev = np.ascontiguousarray(w1.T.reshape(2, 128, 32).transpose(1, 0, 2))
    common = {
        "emb": emb,
        "ident": ident,
        "w1t": w1t_dev,
        "b1": b1.reshape(32, 1),
        "w2l": np.ascontiguousarray(w2.T),
        "b2": b2.reshape(32, 1),
        "w3l": np.ascontiguousarray(w3.T),
        "b3": b3.reshape(1, 1),
    }
    in_maps = []
    for c in range(N_CORES):
        shard = indices[:, c * B_CORE : (c + 1) * B_CORE]
        in_maps.append({**common, "idx": _prep_indices(shard)})

    nc = _get_nc()
    res = run_bass_kernel_spmd(nc, in_maps, core_ids=list(range(N_CORES)))
    ys = [np.asarray(res.results[c]["out"]).reshape(B_CORE) for c in range(N_CORES)]
    return np.concatenate(ys).reshape(BATCH, 1).astype(np.float32)


# revision 29
# speedup vs baseline: 1.2021x; 1.2021x over previous
"""NNUE-style embedding-lookup + tiny-MLP kernel for Trainium2 (8 NeuronCores).

Data-parallel over the batch dim: each of the 8 cores handles 2048 of the
16384 batch positions; the 50 MB embedding table and MLP weights are
replicated per core.

Per-core device program:
  1. dma_gather (SWDGE batched gather) pulls the 32 active-feature embedding
     rows per batch position from HBM into SBUF, 2064 rows per call, rotated
     across 4 SWDGE queues so descriptor generation runs on all Q7 core
     pairs concurrently. Indices are pre-biased by -16384 so they fit the
     gather's signed-int16 index format (the Q7 descriptor generator
     sign-extends and does a signed multiply-accumulate onto the base
     address, so the base is advanced by +16384 rows).
  2. TensorE accumulates the gathered slots (one feature x 128 batches each)
     into PSUM with float32r identity matmuls, two slots (512 columns) per
     matmul; a DVE add folds the even/odd halves into x[128b, 256].
  3. TensorE transposes x, then runs the 256->32->32->1 MLP with ScalarE
     handling bias+ReLU/Tanh.
"""

import numpy as np

INPUT_DIM = 49152
E = 256              # embedding width (1 KB rows)
BATCH = 16384
F = 32               # active features per position
N_CORES = 8
B_CORE = BATCH // N_CORES          # 2048 batch positions per core
BIAS = 16384                       # index bias for int16 gather
CHUNK_REAL = 2048                  # gathered rows per call (16 feats x 128 batches)
PAD = 16                           # tail pad (keeps last index >= 0)
CHUNK = CHUNK_REAL + PAD           # 2064
CW = CHUNK // 16                   # 129 idx cols per chunk in [16, .] layout
CW_STRIDE = 160                    # padded col stride: 320 B, 64 B-aligned
NCH = (B_CORE * F) // CHUNK_REAL   # 32 full-size gather calls (before tail split)
NT = B_CORE // 128                 # 16 t-blocks (128 batches each)
S_REAL = CHUNK_REAL // 128         # 16 real slots per full call
S = (CHUNK + 127) // 128           # 17 slots incl. the pad slot


def _call_table():
    """Per-t-block gather calls: (col_offset, n_cols_used, num_idxs, n_real_slots).

    t-blocks 0..NT-2 use two 2048-row calls; the last t-block is split into
    four 512-row calls so the tail data-drain and compute pipeline finely.
    """
    calls = []
    col = 0
    for t in range(NT):
        sizes = [CHUNK_REAL] if CHUNK_REAL == 4096 else [CHUNK_REAL, CHUNK_REAL]
        tcalls = []
        for sz in sizes:
            n_idx = sz + PAD
            cw = n_idx // 16
            stride = ((cw + 31) // 32) * 32  # 64 B-aligned call starts
            tcalls.append((col, cw, n_idx, sz // 128))
            col += stride
        calls.append(tcalls)
    return calls, col


CALLS, IDX_COLS = _call_table()

_nc_cache = None


def _build():
    import os
    import concourse.bacc as bacc
    import concourse.mybir as mybir
    import concourse.tile as tile

    stage = os.environ.get("KERNEL_STAGE", "full")
    nt = int(os.environ.get("KERNEL_NT", str(NT)))
    bf16 = os.environ.get("KERNEL_BF16", "0") == "1"

    f32 = mybir.dt.float32
    f32r = mybir.dt.float32r
    i16 = mybir.dt.int16
    AF = mybir.ActivationFunctionType

    gdt = mybir.dt.bfloat16 if bf16 else f32r
    edt = mybir.dt.bfloat16 if bf16 else f32
    nc = bacc.Bacc(None, target_bir_lowering=False, debug=False, num_swdge_queues=4)
    emb = nc.dram_tensor("emb", [INPUT_DIM, E], edt, kind="ExternalInput")
    idx = nc.dram_tensor("idx", [128, IDX_COLS], i16, kind="ExternalInput")
    ident = nc.dram_tensor("ident", [128, 128], f32, kind="ExternalInput")
    w1t = nc.dram_tensor("w1t", [128, 2, 32], f32, kind="ExternalInput")
    b1 = nc.dram_tensor("b1", [32, 1], f32, kind="ExternalInput")
    w2l = nc.dram_tensor("w2l", [32, 32], f32, kind="ExternalInput")
    b2 = nc.dram_tensor("b2", [32, 1], f32, kind="ExternalInput")
    w3l = nc.dram_tensor("w3l", [32, 1], f32, kind="ExternalInput")
    b3 = nc.dram_tensor("b3", [1, 1], f32, kind="ExternalInput")
    out = nc.dram_tensor("out", [1, B_CORE], f32, kind="ExternalOutput")

    with tile.TileContext(nc) as tc:
        with (
            tc.tile_pool(name="const", bufs=1) as cpool,
            tc.tile_pool(name="g", bufs=(8 if CHUNK_REAL == 2048 else 5)) as gpool,
            tc.tile_pool(name="xs", bufs=2) as xspool,
            tc.tile_pool(name="xts", bufs=2) as xtspool,
            tc.tile_pool(name="hs", bufs=4) as hspool,
            tc.tile_pool(name="xp", bufs=2, space="PSUM") as xppool,
            tc.tile_pool(name="xtp", bufs=2, space="PSUM") as xtppool,
            tc.tile_pool(name="mp", bufs=4, space="PSUM") as mppool,
        ):
            idx_t = cpool.tile([128, IDX_COLS], i16)
            idx_slice = (IDX_COLS // 8 + 31) // 32 * 32
            for k in range(8):
                lo = k * idx_slice
                hi = min((k + 1) * idx_slice, IDX_COLS)
                if lo < hi:
                    nc.sync.dma_start(idx_t[:, lo:hi], idx[:, lo:hi])
            id_t = cpool.tile([128, 128], f32)
            nc.sync.dma_start(id_t[:], ident[:])
            idr_t = cpool.tile([128, 128], gdt)
            if bf16:
                nc.gpsimd.dma_start(idr_t[:], ident[:])
            else:
                nc.sync.dma_start(idr_t[:], ident[:].bitcast(f32r))
            w1t_t = cpool.tile([128, 2, 32], f32)
            nc.sync.dma_start(w1t_t[:], w1t[:])
            b1_t = cpool.tile([32, 1], f32)
            nc.sync.dma_start(b1_t[:], b1[:])
            w2l_t = cpool.tile([32, 32], f32)
            nc.sync.dma_start(w2l_t[:], w2l[:])
            b2_t = cpool.tile([32, 1], f32)
            nc.sync.dma_start(b2_t[:], b2[:])
            w3l_t = cpool.tile([32, 1], f32)
            nc.sync.dma_start(w3l_t[:], w3l[:])
            b3_t = cpool.tile([1, 1], f32)
            nc.sync.dma_start(b3_t[:], b3[:])
            out_t = cpool.tile([1, B_CORE], f32)

            qn = 0
            for t in range(nt):
                xp = xppool.tile([128, 2, E], f32, tag="xp")
                tcalls = CALLS[t]
                n_mm = sum(nrs // 2 for _, _, _, nrs in tcalls)
                mm = 0
                for col, cw, n_idx, n_real_slots in tcalls:
                    n_slots = (n_idx + 127) // 128
                    g = gpool.tile([128, S, E], gdt, tag="g")
                    nc.gpsimd.dma_gather(
                        g[:, :n_slots, :],
                        emb[BIAS:, :].bitcast(gdt),
                        idx_t[:, col : col + cw],
                        n_idx,
                        n_idx,
                        E,
                        single_packet=False,
                        queue_num=qn % 4,
                    )
                    qn += 1
                    if stage == "gather":
                        nc.sync.dma_start(
                            out[:, (qn % 8) * 128 : (qn % 8) * 128 + 128],
                            g[0:1, 0, :128].bitcast(f32),
                        )
                        continue
                    for sp in range(n_real_slots // 2):
                        nc.tensor.matmul(
                            xp[:],
                            lhsT=idr_t[:],
                            rhs=g[:, 2 * sp : 2 * sp + 2, :],
                            start=(mm == 0),
                            stop=(mm == n_mm - 1),
                        )
                        mm += 1
                if stage == "gather":
                    continue
                x_sb = xspool.tile([128, E], f32, tag="xs")
                nc.vector.tensor_reduce(
                    out=x_sb[:],
                    in_=xp[:].rearrange("p h e -> p e h"),
                    axis=mybir.AxisListType.X,
                    op=mybir.AluOpType.add,
                )
                if stage == "reduce":
                    nc.sync.dma_start(
                        out[:, (t % 8) * 256 : (t % 8) * 256 + 256], x_sb[0:1, :]
                    )
                    continue
                xt_p = xtppool.tile([128, 2, 128], f32, tag="xtp")
                for h in range(2):
                    nc.tensor.transpose(
                        xt_p[:, h, :], x_sb[:, 128 * h : 128 * (h + 1)], id_t[:]
                    )
                xt_sb = xtspool.tile([128, 2, 128], f32, tag="xts")
                nc.vector.tensor_copy(xt_sb[:], xt_p[:])
                h1p = mppool.tile([32, 128], f32, tag="mp")
                for h in range(2):
                    nc.tensor.matmul(
                        h1p[:],
                        lhsT=w1t_t[:, h, :],
                        rhs=xt_sb[:, h, :],
                        start=(h == 0),
                        stop=(h == 1),
                    )
                h1s = hspool.tile([32, 128], f32, tag="hs")
                nc.scalar.activation(h1s[:], h1p[:], AF.Relu, bias=b1_t[:])
                h2p = mppool.tile([32, 128], f32, tag="mp")
                nc.tensor.matmul(h2p[:], lhsT=w2l_t[:], rhs=h1s[:], start=True, stop=True)
                h2s = hspool.tile([32, 128], f32, tag="hs")
                nc.scalar.activation(h2s[:], h2p[:], AF.Relu, bias=b2_t[:])
                yp = mppool.tile([1, 128], f32, tag="mp")
                nc.tensor.matmul(yp[:], lhsT=w3l_t[:], rhs=h2s[:], start=True, stop=True)
                nc.scalar.activation(
                    out_t[:, 128 * t : 128 * (t + 1)], yp[:], AF.Tanh, bias=b3_t[:]
                )
            if stage == "full":
                nc.sync.dma_start(out[:], out_t[:])
    nc.compile()
    return nc


def _get_nc():
    global _nc_cache
    if _nc_cache is None:
        _nc_cache = _build()
    return _nc_cache


def _prep_indices(shard: np.ndarray) -> np.ndarray:
    """[F, B_CORE] int -> [128, NCH*CW_STRIDE] int16 device layout.

    Position order p = t*4096 + f*128 + (b % 128), t = b // 128: each gather
    slot (128 consecutive positions) holds one feature for 128 batches, so
    the feature-sum is a PSUM accumulation over the slots. Each 2064-index
    gather call covers 16 features; indices are biased by -BIAS, padded with
    16 zeros (row BIAS, keeps the tail non-negative so the Q7 truncation
    loop is a no-op), laid out [16, CW] wrapped, replicated across the 8
    Q7 core groups, and 64 B-aligned per call.
    """
    arr = shard.reshape(F, NT, 128)  # [f, t, b_in]
    biased = arr.transpose(1, 0, 2).astype(np.int64) - BIAS  # [t, f, b_in]
    outa = np.zeros((128, IDX_COLS), np.int16)
    for t in range(NT):
        flat = biased[t].reshape(-1)  # 4096 positions, feature-major
        pos = 0
        for col, cw, n_idx, n_real_slots in CALLS[t]:
            n_real = n_real_slots * 128
            lst = np.zeros(n_idx, np.int64)
            lst[:n_real] = flat[pos : pos + n_real]
            pos += n_real
            lay = lst.reshape(cw, 16).T  # [16, cw]
            outa[:, col : col + cw] = np.tile(lay, (8, 1))
    return outa


def kernel(**inputs) -> np.ndarray:
    import os
    from concourse.bass_utils import run_bass_kernel_spmd

    indices = np.asarray(inputs["indices"])
    emb = np.ascontiguousarray(np.asarray(inputs["emb"], dtype=np.float32))
    if os.environ.get("KERNEL_BF16", "0") == "1":
        import ml_dtypes

        emb = emb.astype(ml_dtypes.bfloat16)
    w1 = np.asarray(inputs["w1"], dtype=np.float32)
    b1 = np.asarray(inputs["b1"], dtype=np.float32)
    w2 = np.asarray(inputs["w2"], dtype=np.float32)
    b2 = np.asarray(inputs["b2"], dtype=np.float32)
    w3 = np.asarray(inputs["w3"], dtype=np.float32)
    b3 = np.asarray(inputs["b3"], dtype=np.float32)

    ident = np.eye(128, dtype=np.float32)
    w1t_dev = np.ascontiguousarray(w1.T.reshape(2, 128, 32).transpose(1, 0, 2))
    common = {
        "emb": emb,
        "ident": ident,
        "w1t": w1t_dev,
        "b1": b1.reshape(32, 1),
        "w2l": np.ascontiguousarray(w2.T),
        "b2": b2.reshape(32, 1),
        "w3l": np.ascontiguousarray(w3.T),
        "b3": b3.reshape(1, 1),
    }
    in_maps = []
    for c in range(N_CORES):
        shard = indices[:, c * B_CORE : (c + 1) * B_CORE]
        in_maps.append({**common, "idx": _prep_indices(shard)})

    nc = _get_nc()
    res = run_bass_kernel_spmd(nc, in_maps, core_ids=list(range(N_CORES)))
    ys = [np.asarray(res.results[c]["out"]).reshape(B_CORE) for c in range(N_CORES)]
    return np.concatenate(ys).reshape(BATCH, 1).astype(np.float32)
